# revision 17
# baseline (speedup 1.0000x reference)
"""Trainium2 Bass kernel for a 3D convolution (implicit GEMM).

Problem: B=4, Cin=Cout=64, spatial 32^3, kernel 3^3, stride 1, pad 1, fp32.

Strategy (8 NeuronCores, SPMD):
  - Shard by (batch, z-half): core i handles batch i//2, output planes
    z0 = 16*(i%2) .. z0+16.  Halo of 1 input plane on each side.
  - Host pre-pads each core's input slab to [64, 18, 34, 34] (zero borders
    in y/x and halo planes in z) and casts to bf16.
  - On device, conv = 27 shifted matmuls accumulated in PSUM:
      out[o, pos] = sum_{tap, i} W[o, i, tap] * x[i, pos + shift(tap)]
    with Cin on the SBUF partition axis and flattened (y,x) positions on
    the free axis.  Shifts are pure access-pattern offsets into the padded
    planes; a 2-level free AP (16 rows x stride 34, 32 cols) skips the
    x-padding columns.
  - PE array packing: input planes are split across the two SBUF partition
    halves (planes 0..10 -> partitions 0..64, planes 8..18 -> 64..128), so
    matmuls for low/high output planes use different row groups of the
    128x128 PE array (K=64 each).  Output chunks go to both PSUM partition
    halves (col groups, M=64 each).  Four 64x64 quadrants run concurrently.
  - Each output chunk is a half-plane (z, 16 rows of y), N=512 fp32 = one
    PSUM bank per partition-half; 16 chunks in flight over 8 banks.
  - ScalarE adds the bias while copying PSUM -> SBUF; DMA out as fp32.
"""

import sys

sys.path.insert(0, "/opt/trn_rl_repo")

import numpy as np
import ml_dtypes

import concourse.bass as bass
from concourse import bacc
import concourse.mybir as mybir
from concourse.tile import TileContext
from concourse.bass_utils import run_bass_kernel_spmd

N_CORES = 8
B, C = 4, 64
D = H = W = 32
ZH = 16  # z planes per core
PY, PX = H + 2, W + 2  # padded plane dims
PLANE = PY * PX  # 1156
NPL = 18  # input planes per core incl. halo
HALF_PL = 10  # input planes per partition half
TAPS = [(kd, kh, kw) for kd in range(3) for kh in range(3) for kw in range(3)]

bf16 = mybir.dt.bfloat16
f32 = mybir.dt.float32

_compiled_nc = None


def _build_nc():
    nc = bacc.Bacc()
    xs = nc.dram_tensor("xs", [C, NPL * PLANE], bf16, kind="ExternalInput")
    wq = nc.dram_tensor("wq", [128, 27 * 64], bf16, kind="ExternalInput")
    bias2 = nc.dram_tensor("bias2", [128, 1], f32, kind="ExternalInput")
    out = nc.dram_tensor("out", [C, ZH * H * W], bf16, kind="ExternalOutput")

    with TileContext(nc) as tc:
        with (
            tc.tile_pool(name="const", bufs=1) as cpool,
            tc.tile_pool(name="opool", bufs=2) as opool,
            tc.tile_pool(name="pspool", bufs=8, space="PSUM") as pspool,
        ):
            x_sb = cpool.tile([128, HALF_PL * PLANE], bf16)
            w_sb = cpool.tile([128, 27 * 64], bf16)
            b_sb = cpool.tile([128, 1], f32)
            zz_sb = cpool.tile([128, 640], bf16)
            # The warmup scratch memset goes first so the PE warmup matmuls
            # are not gated on any DMA issue.
            nc.gpsimd.memset(zz_sb[:], 0.0)
            # Chunked input DMAs on the two HWDGE queues (SP carries x half0,
            # ACT carries x half1), ordered by when the matmuls consume them.
            # planes 0..10 -> partitions 0..64 ; planes 8..18 -> partitions 64..128
            nc.gpsimd.dma_start(out=b_sb[:], in_=bias2[:])
            x_chunks = [(0, 2), (2, 4), (4, 6), (6, 8), (8, 10)]
            for ci, (lo, hi) in enumerate(x_chunks):
                nc.sync.dma_start(
                    out=x_sb[0:64, lo * PLANE : hi * PLANE],
                    in_=xs[:, lo * PLANE : hi * PLANE],
                )
                nc.scalar.dma_start(
                    out=x_sb[64:128, lo * PLANE : hi * PLANE],
                    in_=xs[:, (8 + lo) * PLANE : (8 + hi) * PLANE],
                )
                if ci == 0:
                    # First weight chunk (taps 0-4) on ACT right after the
                    # first x chunk; the tail (taps 5-26) on SP.
                    nc.scalar.dma_start(out=w_sb[:, 0 : 5 * 64], in_=wq[:, 0 : 5 * 64])
                    nc.sync.dma_start(out=w_sb[:, 5 * 64 :], in_=wq[:, 5 * 64 :])

            def rhs_ap(rg, off):
                base = x_sb[64 * rg : 64 * rg + 64, off : off + 1]
                return bass.AP(
                    base.tensor, base.offset, [base.ap[0], [PX, 16], [1, W]]
                )

            warm_done = False
            # 4 rounds of 8 chunks; consecutive rounds ping-pong between
            # PSUM bank groups 0-3 / 4-7 so a round's matmuls never wait on
            # the previous round's epilogue.
            for r in range(4):
                psums = [
                    pspool.tile([128, 512], f32, name=f"ps_{r}_{b_}", tag="ps")
                    for b_ in range(4)
                ]
                if not warm_done:
                    # Warm up the PE HAM clock gate while the x DMAs stream:
                    # ~3.5us of full-array matmuls on zeroed scratch data so
                    # the real matmuls start at 2.4 GHz instead of 1.2 GHz.
                    warm_done = True
                    for _ in range(20):
                        nc.tensor.matmul(
                            psums[0][:, :],
                            zz_sb[:, 0:128],
                            zz_sb[:, 128:640],
                            start=True,
                            stop=True,
                            skip_group_check=True,
                        )
                for t, (kd, kh, kw) in enumerate(TAPS):
                    for rg in (0, 1):  # row group == partition half of x
                        lhsT = w_sb[64 * rg : 64 * rg + 64, t * 64 : t * 64 + 64]
                        for cg in (0, 1):  # col group == psum partition half
                            for z2 in range(2):
                                off = (
                                    (r * 2 + z2 + kd) * PLANE
                                    + (16 * cg + kh) * PY
                                    + kw
                                )
                                nc.tensor.matmul(
                                    psums[rg * 2 + z2][64 * cg : 64 * cg + 64, :],
                                    lhsT,
                                    rhs_ap(rg, off),
                                    start=(t == 0),
                                    stop=(t == 26),
                                    skip_group_check=True,
                                )
                # Stage the round's output in SBUF (bias added while copying,
                # split across ScalarE and VectorE), then strided DMAs.
                o_sb = opool.tile([128, 4 * 512], bf16, name=f"o_{r}", tag="o")
                for bank in range(4):
                    dst_sl = o_sb[:, bank * 512 : (bank + 1) * 512]
                    if bank % 2 == 0:
                        nc.scalar.activation(
                            dst_sl,
                            psums[bank][:],
                            mybir.ActivationFunctionType.Identity,
                            bias=b_sb[:],
                        )
                    else:
                        nc.vector.tensor_scalar_add(dst_sl, psums[bank][:], b_sb[:])
                for rg in (0, 1):
                    eng = nc.sync if rg == 0 else nc.scalar
                    for cg in (0, 1):
                        dst0 = (rg * 8 + r * 2) * 1024 + 512 * cg
                        base = out[:, dst0 : dst0 + 1]
                        dst = bass.AP(
                            base.tensor, base.offset, [base.ap[0], [1024, 2], [1, 512]]
                        )
                        eng.dma_start(
                            out=dst,
                            in_=o_sb[
                                64 * cg : 64 * cg + 64,
                                rg * 2 * 512 : (rg * 2 + 2) * 512,
                            ],
                        )
    nc.compile()
    return nc


def _get_nc():
    global _compiled_nc
    if _compiled_nc is None:
        _compiled_nc = _build_nc()
    return _compiled_nc


def _prepare_inputs(x, weight, bias):
    x = np.asarray(x, dtype=np.float32)
    weight = np.asarray(weight, dtype=np.float32)
    bias = np.asarray(bias, dtype=np.float32).reshape(C)

    # weights: [cout, cin, kd, kh, kw] -> [cin, tap, cout], duplicated into
    # both partition halves: [128 partitions, 27*64]
    wt = weight.transpose(1, 2, 3, 4, 0).reshape(C, 27, C)
    wq = np.empty((128, 27, C), dtype=np.float32)
    wq[0:64] = wt
    wq[64:128] = wt
    wq = wq.reshape(128, 27 * 64).astype(ml_dtypes.bfloat16)

    bias2 = np.empty((128, 1), dtype=np.float32)
    bias2[0:64, 0] = bias
    bias2[64:128, 0] = bias

    in_maps = []
    for i in range(N_CORES):
        b, h = i // 2, i % 2
        z0 = ZH * h
        slab = np.zeros((C, NPL, PY, PX), dtype=np.float32)
        zlo_g, zhi_g = max(0, z0 - 1), min(D, z0 + ZH + 1)
        slab[:, zlo_g - (z0 - 1) : zhi_g - (z0 - 1), 1 : H + 1, 1 : W + 1] = x[
            b, :, zlo_g:zhi_g
        ]
        in_maps.append(
            {
                "xs": slab.reshape(C, NPL * PLANE).astype(ml_dtypes.bfloat16),
                "wq": wq,
                "bias2": bias2,
            }
        )
    return in_maps


def _run(in_maps, trace=False):
    nc = _get_nc()
    return run_bass_kernel_spmd(
        nc, in_maps, core_ids=list(range(N_CORES)), trace=trace
    )


def kernel(x, weight, bias):
    in_maps = _prepare_inputs(x, weight, bias)
    res = _run(in_maps, trace=False)
    out = np.empty((B, C, D, H, W), dtype=np.float32)
    for i in range(N_CORES):
        b, h = i // 2, i % 2
        z0 = ZH * h
        out[b, :, z0 : z0 + ZH] = res.results[i]["out"].astype(np.float32).reshape(
            C, ZH, H, W
        )
    return out


# revision 18
# speedup vs baseline: 1.2078x; 1.2078x over previous
"""Trainium2 Bass kernel for a 3D convolution (implicit GEMM).

Problem: B=4, Cin=Cout=64, spatial 32^3, kernel 3^3, stride 1, pad 1, fp32.

Strategy (8 NeuronCores, SPMD):
  - Shard by (batch, z-half): core i handles batch i//2, output planes
    z0 = 16*(i%2) .. z0+16.  Halo of 1 input plane on each side.
  - Host pre-pads each core's input slab to [64, 18, 34, 34] (zero borders
    in y/x and halo planes in z) and casts to bf16.
  - On device, conv = 27 shifted matmuls accumulated in PSUM:
      out[o, pos] = sum_{tap, i} W[o, i, tap] * x[i, pos + shift(tap)]
    with Cin on the SBUF partition axis and flattened (y,x) positions on
    the free axis.  Shifts are pure access-pattern offsets into the padded
    planes; a 2-level free AP (16 rows x stride 34, 32 cols) skips the
    x-padding columns.
  - PE array packing: input planes are split across the two SBUF partition
    halves (planes 0..10 -> partitions 0..64, planes 8..18 -> 64..128), so
    matmuls for low/high output planes use different row groups of the
    128x128 PE array (K=64 each).  Output chunks go to both PSUM partition
    halves (col groups, M=64 each).  Four 64x64 quadrants run concurrently.
  - Each output chunk is a half-plane (z, 16 rows of y), N=512 fp32 = one
    PSUM bank per partition-half; 16 chunks in flight over 8 banks.
  - ScalarE adds the bias while copying PSUM -> SBUF; DMA out as fp32.
"""

import sys

sys.path.insert(0, "/opt/trn_rl_repo")

import numpy as np
import ml_dtypes

import concourse.bass as bass
from concourse import bacc
import concourse.mybir as mybir
from concourse.tile import TileContext
from concourse.bass_utils import run_bass_kernel_spmd

N_CORES = 8
B, C = 4, 64
D = H = W = 32
ZH = 16  # z planes per core
PY, PX = H + 2, W + 2  # padded plane dims
PLANE = PY * PX  # 1156
NPL = 18  # input planes per core incl. halo
HALF_PL = 10  # input planes per partition half
TAPS = [(kd, kh, kw) for kd in range(3) for kh in range(3) for kw in range(3)]

bf16 = mybir.dt.bfloat16
f32 = mybir.dt.float32

_compiled_nc = None


def _build_nc():
    nc = bacc.Bacc()
    xs = nc.dram_tensor("xs", [C, NPL * PLANE], bf16, kind="ExternalInput")
    wq = nc.dram_tensor("wq", [128, 27 * 64], bf16, kind="ExternalInput")
    bias2 = nc.dram_tensor("bias2", [128, 1], f32, kind="ExternalInput")
    out = nc.dram_tensor("out", [C, ZH * H * W], bf16, kind="ExternalOutput")

    with TileContext(nc) as tc:
        with (
            tc.tile_pool(name="const", bufs=1) as cpool,
            tc.tile_pool(name="opool", bufs=2) as opool,
            tc.tile_pool(name="pspool", bufs=8, space="PSUM") as pspool,
        ):
            x_sb = cpool.tile([128, HALF_PL * PLANE], bf16)
            w_sb = cpool.tile([128, 27 * 64], bf16)
            b_sb = cpool.tile([128, 1], f32)
            zz_sb = cpool.tile([128, 640], bf16)
            # The warmup scratch memset goes first so the PE warmup matmuls
            # are not gated on any DMA issue.
            nc.gpsimd.memset(zz_sb[:], 0.0)
            # Chunked input DMAs on the two HWDGE queues (SP carries x half0,
            # ACT carries x half1), ordered by when the matmuls consume them.
            # planes 0..10 -> partitions 0..64 ; planes 8..18 -> partitions 64..128
            nc.gpsimd.dma_start(out=w_sb[:, 0 : 5 * 64], in_=wq[:, 0 : 5 * 64])
            nc.gpsimd.dma_start(out=b_sb[:], in_=bias2[:])
            x_chunks = [(0, 2), (2, 4), (4, 6), (6, 8), (8, 10)]
            for ci, (lo, hi) in enumerate(x_chunks):
                nc.sync.dma_start(
                    out=x_sb[0:64, lo * PLANE : hi * PLANE],
                    in_=xs[:, lo * PLANE : hi * PLANE],
                )
                nc.scalar.dma_start(
                    out=x_sb[64:128, lo * PLANE : hi * PLANE],
                    in_=xs[:, (8 + lo) * PLANE : (8 + hi) * PLANE],
                )
                if ci == 0:
                    # Weight tail rides the SP queue right after the first
                    # x chunk (needed from ~tap 5 of round 0).
                    nc.sync.dma_start(out=w_sb[:, 5 * 64 :], in_=wq[:, 5 * 64 :])

            def rhs_ap(rg, off):
                base = x_sb[64 * rg : 64 * rg + 64, off : off + 1]
                return bass.AP(
                    base.tensor, base.offset, [base.ap[0], [PX, 16], [1, W]]
                )

            warm_done = False
            # 4 rounds of 8 chunks; consecutive rounds ping-pong between
            # PSUM bank groups 0-3 / 4-7 so a round's matmuls never wait on
            # the previous round's epilogue.
            for r in range(4):
                psums = [
                    pspool.tile([128, 512], f32, name=f"ps_{r}_{b_}", tag="ps")
                    for b_ in range(4)
                ]
                if not warm_done:
                    # Warm up the PE HAM clock gate while the x DMAs stream:
                    # ~3.5us of full-array matmuls on zeroed scratch data so
                    # the real matmuls start at 2.4 GHz instead of 1.2 GHz.
                    warm_done = True
                    for _ in range(9):
                        nc.tensor.matmul(
                            psums[0][:, :],
                            zz_sb[:, 0:128],
                            zz_sb[:, 128:640],
                            start=True,
                            stop=True,
                            skip_group_check=True,
                        )
                for t, (kd, kh, kw) in enumerate(TAPS):
                    for rg in (0, 1):  # row group == partition half of x
                        lhsT = w_sb[64 * rg : 64 * rg + 64, t * 64 : t * 64 + 64]
                        for cg in (0, 1):  # col group == psum partition half
                            for z2 in range(2):
                                off = (
                                    (r * 2 + z2 + kd) * PLANE
                                    + (16 * cg + kh) * PY
                                    + kw
                                )
                                nc.tensor.matmul(
                                    psums[rg * 2 + z2][64 * cg : 64 * cg + 64, :],
                                    lhsT,
                                    rhs_ap(rg, off),
                                    start=(t == 0),
                                    stop=(t == 26),
                                    skip_group_check=True,
                                )
                # Stage the round's output in SBUF (bias added while copying,
                # split across ScalarE and VectorE), then strided DMAs.
                o_sb = opool.tile([128, 4 * 512], bf16, name=f"o_{r}", tag="o")
                for bank in range(4):
                    dst_sl = o_sb[:, bank * 512 : (bank + 1) * 512]
                    if bank % 2 == 0:
                        nc.scalar.activation(
                            dst_sl,
                            psums[bank][:],
                            mybir.ActivationFunctionType.Identity,
                            bias=b_sb[:],
                        )
                    else:
                        nc.vector.tensor_scalar_add(dst_sl, psums[bank][:], b_sb[:])
                for rg in (0, 1):
                    eng = nc.sync if rg == 0 else nc.scalar
                    for cg in (0, 1):
                        dst0 = (rg * 8 + r * 2) * 1024 + 512 * cg
                        base = out[:, dst0 : dst0 + 1]
                        dst = bass.AP(
                            base.tensor, base.offset, [base.ap[0], [1024, 2], [1, 512]]
                        )
                        eng.dma_start(
                            out=dst,
                            in_=o_sb[
                                64 * cg : 64 * cg + 64,
                                rg * 2 * 512 : (rg * 2 + 2) * 512,
                            ],
                        )
    nc.compile()
    return nc


def _get_nc():
    global _compiled_nc
    if _compiled_nc is None:
        _compiled_nc = _build_nc()
    return _compiled_nc


def _prepare_inputs(x, weight, bias):
    x = np.asarray(x, dtype=np.float32)
    weight = np.asarray(weight, dtype=np.float32)
    bias = np.asarray(bias, dtype=np.float32).reshape(C)

    # weights: [cout, cin, kd, kh, kw] -> [cin, tap, cout], duplicated into
    # both partition halves: [128 partitions, 27*64]
    wt = weight.transpose(1, 2, 3, 4, 0).reshape(C, 27, C)
    wq = np.empty((128, 27, C), dtype=np.float32)
    wq[0:64] = wt
    wq[64:128] = wt
    wq = wq.reshape(128, 27 * 64).astype(ml_dtypes.bfloat16)

    bias2 = np.empty((128, 1), dtype=np.float32)
    bias2[0:64, 0] = bias
    bias2[64:128, 0] = bias

    in_maps = []
    for i in range(N_CORES):
        b, h = i // 2, i % 2
        z0 = ZH * h
        slab = np.zeros((C, NPL, PY, PX), dtype=np.float32)
        zlo_g, zhi_g = max(0, z0 - 1), min(D, z0 + ZH + 1)
        slab[:, zlo_g - (z0 - 1) : zhi_g - (z0 - 1), 1 : H + 1, 1 : W + 1] = x[
            b, :, zlo_g:zhi_g
        ]
        in_maps.append(
            {
                "xs": slab.reshape(C, NPL * PLANE).astype(ml_dtypes.bfloat16),
                "wq": wq,
                "bias2": bias2,
            }
        )
    return in_maps


def _run(in_maps, trace=False):
    nc = _get_nc()
    return run_bass_kernel_spmd(
        nc, in_maps, core_ids=list(range(N_CORES)), trace=trace
    )


def kernel(x, weight, bias):
    in_maps = _prepare_inputs(x, weight, bias)
    res = _run(in_maps, trace=False)
    out = np.empty((B, C, D, H, W), dtype=np.float32)
    for i in range(N_CORES):
        b, h = i // 2, i % 2
        z0 = ZH * h
        out[b, :, z0 : z0 + ZH] = res.results[i]["out"].astype(np.float32).reshape(
            C, ZH, H, W
        )
    return out
